# revision 35
# baseline (speedup 1.0000x reference)
"""MHSA Trainium2 kernel v2: B=2, N=2048, H=1024, 16 heads x d=64.

Sharding: 8 cores = 2 (batch) x 4 (head-groups of 4 heads); cores fully
independent, host gathers.

Per-core plan (cost model: matmul = out_free x pe_cycle x cpr, fp8-DR cpr=0.5;
ACT exp = free x 0.833ns and is the bottleneck at ~133us):
  - projections in bf16 (hs, W bf16 host-side): Q,K -> PSUM acc [128, 512]
    per (col-chunk, token-tile), DVE-converted to fp8 e4m3 into QT8/KT8
    [128, 2, 4h, 2048] "DoubleRow" layout: head-even d-rows on partitions
    0-63 slot 0, head-odd on 64-127 slot 0, other slot zero; row 64 carries
    an augmentation: ones (Q side) and mask-bias 0/-224 (K side) in slot
    h%2, so no per-key bias is needed at exp time.
  - scores^T per (head, jt, ib): fp8 DoubleRow matmuls (2 x 512-free) into
    psum [128, 1024]; ACT exp (scale=1/32, bias=0) -> PT bf16 quads
    [128, 8jt, 2048].
  - attnV orientation B: out[i-chunk 128, 65] += PT[:, jt, ic]^T @
    V_aug[:, jt, h, :] accumulating over 16 jt in one psum bank; col 64
    (ones col of V_aug) accumulates the softmax denominator. DVE reciprocal
    + per-partition tensor_scalar multiply -> staged [128, 16, 64] f32,
    one DMA per head to out[2048, 256].
  - emission interleaves proj(h+1)/attnV(h-1)/V-proj under head-h's exp
    stream so ACT never idles.
"""

import numpy as np
import ml_dtypes

import concourse.bass as bass
import concourse.bacc as bacc
import concourse.mybir as mybir
import concourse.tile as tile
from concourse.bass_utils import run_bass_kernel_spmd

F32 = mybir.dt.float32
BF16 = mybir.dt.bfloat16
F8 = mybir.dt.float8e4
AF = mybir.ActivationFunctionType
DR = mybir.MatmulPerfMode.DoubleRow

HID = 1024
NT = 2048
D = 64
HPC = 4  # heads per core
NCORES = 8
SCALE = float(HID) ** -0.5  # 1/32
KD = HID // 128  # 8 contraction chunks
NJT = NT // 128  # 16 key tiles
NIC = NT // 128  # 16 query chunks (attnV orientation B)
MASK_BIAS = -224.0  # exp(SCALE*-224) = e^-7 ~ 9e-4; exactly representable e4m3

_CACHE = {}


def _build():
    if "nc" in _CACHE:
        return _CACHE["nc"]
    nc = bacc.Bacc("TRN2", debug=False)
    hsb_d = nc.dram_tensor("hsb", [128, KD, NT], BF16, kind="ExternalInput")
    wqk_d = nc.dram_tensor("wqk", [128, KD, 8 * D], BF16, kind="ExternalInput")
    wv_d = nc.dram_tensor("wv", [128, KD, HPC * D], BF16, kind="ExternalInput")
    augq_d = nc.dram_tensor("augq", [1, NT], F8, kind="ExternalInput")
    augk_d = nc.dram_tensor("augk", [1, NT], F8, kind="ExternalInput")
    out_d = nc.dram_tensor("out", [NT, HPC * D], F32, kind="ExternalOutput")

    with tile.TileContext(nc) as tc:
        with tc.tile_pool(name="per", bufs=1) as per:
            hsb = per.tile([128, KD, NT], BF16, tag="hsb")
            wqk = per.tile([128, KD, 8 * D], BF16, tag="wqk")
            wv = per.tile([128, KD, HPC * D], BF16, tag="wv")
            QT8 = per.tile([128, 2, HPC, NT], F8, tag="qt8")
            KT8 = per.tile([128, 2, HPC, NT], F8, tag="kt8")
            Vall = per.tile([128, NJT, HPC, D + 1], BF16, tag="vall")
            PTq = [per.tile([128, 8, NT], BF16, tag=f"ptq{i}", name=f"ptq{i}") for i in range(3)]
            stage = [per.tile([128, NIC, D], F32, tag=f"stg{i}", name=f"stg{i}") for i in range(2)]
            rlp = [per.tile([128, 1], F32, tag=f"rl{i}", name=f"rl{i}") for i in range(4)]
            vosb = [per.tile([128, D + 1], F32, tag=f"vs{i}", name=f"vs{i}") for i in range(2)]

            # input DMAs, priority order: the transfers serialize at
            # aggregate DMA bandwidth, so ship only what the prologue needs
            # first (wqk q01/k01 half, hs tokens 0-1023).
            # fp8 Q/K tiles zero on Pool (4x via u32 bitcast) first so the
            # aug-row DMA waits resolve early.
            U32 = mybir.dt.uint32
            nc.gpsimd.memset(KT8[:].bitcast(U32), 0)
            nc.gpsimd.memset(QT8[:].bitcast(U32), 0)
            nc.gpsimd.memset(Vall[:, :, :, D : D + 1], 1.0)
            # Input DMAs, priority order: transfers serialize at aggregate
            # DMA bandwidth, so ship what the prologue needs first (wqk
            # q01/k01 half, hs tokens 0-1023), and slot the tiny aug-row
            # DMAs (Q: ones, K: mask bias at partition 64, slot h%2, per
            # head, no overlap with the proj convert regions) in the middle
            # so they reach the transfer queue before the first scores.
            nc.sync.dma_start(out=wqk[:, :, 0:256], in_=wqk_d.ap()[:, :, 0:256])
            nc.sync.dma_start(out=hsb[:, :, 0:512], in_=hsb_d.ap()[:, :, 0:512])
            nc.sync.dma_start(out=hsb[:, :, 512:1024], in_=hsb_d.ap()[:, :, 512:1024])
            for h in range(HPC):
                nc.sync.dma_start(out=KT8[64:65, h % 2, h, :], in_=augk_d.ap())
            for h in range(HPC):
                nc.sync.dma_start(out=QT8[64:65, h % 2, h, :], in_=augq_d.ap())
            nc.sync.dma_start(out=wv[:], in_=wv_d.ap())
            nc.sync.dma_start(out=hsb[:, :, 1024:1536], in_=hsb_d.ap()[:, :, 1024:1536])
            nc.sync.dma_start(out=hsb[:, :, 1536:2048], in_=hsb_d.ap()[:, :, 1536:2048])
            nc.sync.dma_start(out=wqk[:, :, 256:512], in_=wqk_d.ap()[:, :, 256:512])

            with (
                tc.tile_pool(name="psc", bufs=2, space="PSUM") as psc,
                tc.tile_pool(name="pw1", bufs=2, space="PSUM") as pw1,
                tc.tile_pool(name="ppv", bufs=2, space="PSUM") as ppv,
            ):
                lp = nc.allow_low_precision
                RSN = "fp8/bf16 quantization is intentional"

                def qk_unit(c, t):
                    # wqk col chunks: 0=q_h0|q_h1 1=k_h0|k_h1 2=q_h2|q_h3 3=k_h2|k_h3
                    acc = pw1.tile([128, 512], F32, tag="w1", name=f"acc{c}_{t}")
                    for k in range(KD):
                        nc.tensor.matmul(
                            acc[:],
                            wqk[:, k, c * 128 : (c + 1) * 128],
                            hsb[:, k, t * 512 : (t + 1) * 512],
                            start=(k == 0),
                            stop=(k == KD - 1),
                        )
                    dst = QT8 if c % 2 == 0 else KT8
                    h0 = (c // 2) * 2  # heads h0 (even) and h0+1 (odd)
                    ts = slice(t * 512, (t + 1) * 512)
                    with lp(RSN):
                        nc.vector.tensor_copy(dst[0:64, 0, h0, ts], acc[0:64, :])
                        nc.vector.tensor_copy(
                            dst[64:128, 0, h0 + 1, ts], acc[64:128, :]
                        )

                def v_unit(t):
                    pv = ppv.tile([128, HPC * D], F32, tag="pv", name=f"pv{t}")
                    for k in range(KD):
                        nc.tensor.matmul(
                            pv[:],
                            hsb[:, k, t * 128 : (t + 1) * 128],
                            wv[:, k, :],
                            start=(k == 0),
                            stop=(k == KD - 1),
                        )
                    with lp(RSN):
                        for hh in range(HPC):
                            nc.vector.tensor_copy(
                                Vall[:, t, hh, 0:D], pv[:, hh * D : (hh + 1) * D]
                            )

                def scores_exp(h, jt, i0, i1):
                    # high priority: the exp stream is the kernel bottleneck;
                    # never let the list scheduler hoist side work before the
                    # score matmuls feeding it.
                    with tc.high_priority(offset=120):
                        n = i1 - i0
                        sc = psc.tile([128, 1024], F32, tag="sc", name=f"sc{h}_{jt}_{i0}")
                        for ic in range(n // 512):
                            c0 = i0 + ic * 512
                            nc.tensor.matmul(
                                sc[:, ic * 512 : (ic + 1) * 512],
                                KT8[:, :, h, jt * 128 : (jt + 1) * 128],
                                QT8[:, :, h, c0 : c0 + 512],
                                start=True,
                                stop=True,
                                perf_mode=DR,
                            )
                        q, slot = divmod(jt, 8)
                        q = (2 * h + q) % 3
                        with lp(RSN):
                            nc.scalar.activation(
                                PTq[q][:, slot, i0:i1],
                                sc[:, 0:n],
                                AF.Exp,
                                bias=0.0,
                                scale=SCALE,
                            )

                def attnv_unit(h, ic, pool, tag):
                    vo = pool.tile([128, 512], F32, tag=tag, name=f"vo{h}_{ic}")
                    for jt in range(NJT):
                        q, slot = divmod(jt, 8)
                        q = (2 * h + q) % 3
                        nc.tensor.matmul(
                            vo[:, 0 : D + 1],
                            PTq[q][:, slot, ic * 128 : (ic + 1) * 128],
                            Vall[:, jt, h, :],
                            start=(jt == 0),
                            stop=(jt == NJT - 1),
                        )
                    rl = rlp[ic % 4]
                    with lp(RSN):
                        nc.vector.reciprocal(rl[:], vo[:, D : D + 1])
                        nc.vector.tensor_scalar_mul(
                            stage[h % 2][:, ic, :], vo[:, 0:D], rl[:]
                        )

                def head_out_dma(h):
                    nc.sync.dma_start(
                        out=out_d.ap().rearrange("(c p) m -> p c m", p=128)[
                            :, :, h * D : (h + 1) * D
                        ],
                        in_=stage[h % 2][:],
                    )

                # PE warm-up: the cost model runs matmuls 2-4x slower until
                # the tensor engine has been continuously busy for 3us, and
                # an idle gap resets the ramp. Grind dummy matmuls on a
                # zeroed scratch tile while the first hs chunks arrive so
                # the real prologue runs at full clock.
                dumm = per.tile([128, 256], BF16, tag="dumm")
                nc.vector.memset(dumm[:].bitcast(U32), 0)
                dacc = pw1.tile([128, 512], F32, tag="w1", name="dacc")
                for i in range(36):
                    nc.tensor.matmul(
                        dacc[:, 0:256],
                        dumm[:, 0:128],
                        dumm[:],
                        start=True,
                        stop=True,
                    )

                # prologue: enough of heads 0/1 q+k proj for the first groups
                qk_unit(1, 0)  # k_h0|k_h1 tokens 0-511 (key tiles 0-3)
                qk_unit(0, 0)  # q_h0|q_h1 tokens 0-511
                qk_unit(0, 1)  # q tokens 512-1023 -> i-block 0 complete

                # side-work tables: side[h][slot] emitted after that slot's
                # scores+exp. 32 slots per head window. h0 runs ib-outer
                # (slot = ib*16+jt) so early slots only need kA-t0/qA-t0,t1;
                # h1-h3 run ib-inner (slot = jt*2+ib) so PT quads written in
                # jt order, letting the 3-quad rotation reuse safely.
                side = [[[] for _ in range(32)] for _ in range(4)]
                for i, (c, t) in enumerate([(1, 1), (1, 2), (1, 3), (0, 2), (0, 3)]):
                    side[0][2 * i].append((qk_unit, (c, t)))
                # v-proj ends by slot 29 so the h0->h1 transition has no
                # PE work between the last h0 scores and the first h1 scores
                for t in range(14):
                    side[0][2 * t + 1].append((v_unit, (t,)))
                side[0][28].append((v_unit, (14,)))
                side[0][29].append((v_unit, (15,)))
                # heads 2,3 q/k proj in h1's second half (kB0,qB0..3 first
                # so h2's ib-inner slot 0/1 have full i-range and jt0-3)
                for i, (c, t) in enumerate(
                    [(3, 0), (2, 0), (2, 1), (2, 2), (2, 3), (3, 1), (3, 2), (3, 3)]
                ):
                    side[1][14 + 2 * i].append((qk_unit, (c, t)))
                # attnV of head h-1 front-loaded in head h's window so the
                # 3-quad PT rotation frees quads before h's second half
                for h in range(1, 4):
                    side[h][0].append((attnv_unit, (h - 1, 0, pw1, "w1")))
                    side[h][0].append((attnv_unit, (h - 1, 1, pw1, "w1")))
                    for ic in range(2, 16):
                        side[h][ic - 1].append((attnv_unit, (h - 1, ic, pw1, "w1")))
                    side[h][16].append((head_out_dma, (h - 1,)))

                # Pin side units to their intended slot times so the list
                # scheduler (whose internal DMA timing is optimistic) cannot
                # hoist them ahead of the exp-critical chain.
                T0, SLOT = 0.0125, 0.00104  # ms
                for h in range(HPC):
                    if h == 0:
                        order = [
                            (jt, ib * 1024, (ib + 1) * 1024)
                            for ib in range(2)
                            for jt in range(NJT)
                        ]
                    else:
                        order = [
                            (jt, ib * 1024, (ib + 1) * 1024)
                            for jt in range(NJT)
                            for ib in range(2)
                        ]
                    for s, (jt, i0, i1) in enumerate(order):
                        scores_exp(h, jt, i0, i1)
                        with tc.tile_wait_until(
                            T0 + (32 * h + s + 0.5) * SLOT, enable=(h == 0)
                        ):
                            for fn, args in side[h][s]:
                                fn(*args)

                # tail: head 3 attnV. Exposed after the last exp, so keep it
                # matmul-bound: vouts rotate across both psum pools and the
                # normalize runs as DVE copy + Pool normalize_recip instead
                # of the longer-latency DVE recip+mul chain.
                for ic in range(16):
                    pool, tag = (psc, "sc") if ic % 2 else (pw1, "w1")
                    vo = pool.tile([128, 512], F32, tag=tag, name=f"vot{ic}")
                    for jt in range(NJT):
                        q, slot = divmod(jt, 8)
                        q = (2 * 3 + q) % 3
                        nc.tensor.matmul(
                            vo[:, 0 : D + 1],
                            PTq[q][:, slot, ic * 128 : (ic + 1) * 128],
                            Vall[:, jt, 3, :],
                            start=(jt == 0),
                            stop=(jt == NJT - 1),
                        )
                    vs = vosb[ic % 2]
                    with lp(RSN):
                        nc.vector.tensor_copy(vs[:], vo[:, 0 : D + 1])
                        nc.gpsimd.normalize_recip(
                            stage[1][:, ic, :], vs[:, 0:D], vs[:, D : D + 1]
                        )
                    if ic % 4 == 3:
                        # overlap the output DMA with the remaining units
                        nc.sync.dma_start(
                            out=out_d.ap().rearrange("(c p) m -> p c m", p=128)[
                                :, ic - 3 : ic + 1, 3 * D : 4 * D
                            ],
                            in_=stage[1][:, ic - 3 : ic + 1, :],
                        )

    if not nc.is_finalized():
        nc.finalize()
    _CACHE["nc"] = nc
    return nc


def kernel(hidden_states, attention_mask, W_qkv):
    hs = np.asarray(hidden_states, dtype=np.float32)  # [2, 2048, 1024]
    am = np.asarray(attention_mask)  # [2, 2048]
    W = np.asarray(W_qkv, dtype=np.float32)  # [16, 1024, 192]

    nc = _build()
    bf = ml_dtypes.bfloat16
    f8 = ml_dtypes.float8_e4m3
    in_maps = []
    for core in range(NCORES):
        b, hg = core // 4, core % 4
        Wc = W[hg * HPC : (hg + 1) * HPC]  # [4, 1024, 192]
        hsT = np.ascontiguousarray(hs[b].T)  # [1024, 2048]
        hsb = np.ascontiguousarray(
            hsT.reshape(KD, 128, NT).transpose(1, 0, 2)
        ).astype(bf)
        qcols = [Wc[h, :, 0:D] for h in range(HPC)]
        kcols = [Wc[h, :, D : 2 * D] for h in range(HPC)]
        vcols = [Wc[h, :, 2 * D : 3 * D] for h in range(HPC)]
        # col chunks of 128: [q_h0|q_h1, k_h0|k_h1, q_h2|q_h3, k_h2|k_h3]
        wqk = np.concatenate(
            qcols[0:2] + kcols[0:2] + qcols[2:4] + kcols[2:4], axis=1
        )  # [1024, 512]
        wvv = np.concatenate(vcols, axis=1)  # [1024, 256]
        wqk8 = np.ascontiguousarray(
            wqk.reshape(KD, 128, 8 * D).transpose(1, 0, 2)
        ).astype(bf)
        wv8 = np.ascontiguousarray(
            wvv.reshape(KD, 128, HPC * D).transpose(1, 0, 2)
        ).astype(bf)
        bias = np.where(am[b] != 0, 0.0, MASK_BIAS).astype(np.float32)  # [2048]
        augq = np.ones((1, NT), dtype=f8)
        augk = bias.astype(f8).reshape(1, NT)
        in_maps.append(
            {
                "hsb": hsb,
                "wqk": wqk8,
                "wv": wv8,
                "augq": augq,
                "augk": augk,
            }
        )
    res = run_bass_kernel_spmd(nc, in_maps, list(range(NCORES)))
    if res.exec_time_ns is not None:
        print(f"HW exec time: {res.exec_time_ns} ns")
    if res.mean_exec_time_ns is not None:
        print(f"HW exec time (mean across cores): {res.mean_exec_time_ns} ns")
    out = np.empty((2, NT, HID), dtype=np.float32)
    for core in range(NCORES):
        b, hg = core // 4, core % 4
        out[b, :, hg * 256 : (hg + 1) * 256] = res.results[core]["out"]
    return out


def predicted_exec_ns():
    """Device-occupancy estimate for one core (all 8 run the same program
    in parallel)."""
    nc = _build()
    from concourse.timeline_sim import TimelineSim
    return float(TimelineSim(nc, trace=False).simulate())
